# revision 31
# baseline (speedup 1.0000x reference)
"""Trainium2 Bass kernel for an 8-expert top-2 MoE layer (+ shared expert).

Single-NEFF fused design over 8 NeuronCores. The router (0.03% of FLOPs)
and the top-2 dispatch/combine run on the host in exact fp32; each core
runs ONE program computing

  - the shared-expert FFN for its 512-token slice (fp8 e4m3 residual
    matmuls, 3 terms/layer: w_hi.x_hi + w_lo.x_hi + w_hi.x_lo), and
  - expert j's FFN over C=1024 routed tokens, gathered by the host and
    sorted by combine weight descending. Slots use a precision ladder:
    the 128 highest-weight slots get a 2-term scheme per layer (both
    weight planes vs activation-hi), the rest run 1-term (hi-only).
    Routed activations never need a residual split on device.

All matmuls are fp8(e4m3) DoubleRow (two 128-deep k-tiles per
instruction, fp32 PSUM). Per-expert token lists beyond the capacity C
fall back to an exact host fp32 path (the dropped tokens are the
lowest-combine-weight ones, so this also improves accuracy).

Perf structure: one NEFF = one p-state ramp + one startup/drain. Weight
buffers are reused in place: the shared-expert w1 chunk tiles are
overwritten by expert w1 chunks as soon as shared layer 1 has consumed
them (fine-grained per-chunk WAR), likewise sw2 -> w2 halves behind
shared layer 2. Routed layer-1 tiles are interleaved into the shared
layer-2 (and routed layer-2) instruction stream so the PE never waits on
a gelu chain; dummy warm-up matmuls carry the PE through its p-state
ramp during the startup DMAs. The combine-weight multiply and the
scatter-add happen on the host.
"""

import sys

if "/opt/trn_rl_repo" not in sys.path:
    sys.path.insert(0, "/opt/trn_rl_repo")

import math

import numpy as np
import ml_dtypes

DIM = 1024
E = 8
H = 4096
T = 4096  # B*S = 2*2048 tokens
NCORES = 8
P = 128
DKO = DIM // P     # 8 k-subtiles over dim
HKO = H // P       # 32 k-subtiles over hidden
TS = T // NCORES   # 512 shared tokens per core
CAP = 1024         # routed capacity per expert (overflow -> exact host path)
HI = 128           # slots with the 2-term scheme (highest combine weight)
RSL = [512, 512]   # routed token slices
XA = 384           # shared-x split: big piece lands first, its L1 compute
XB = TS - XA       # covers the small piece's arrival

E4M3 = ml_dtypes.float8_e4m3
F32 = np.float32

_nc_cache = {}


def _split_fp8(a):
    """e4m3 hi/lo residual split (same scale for both planes)."""
    hi = np.asarray(a, E4M3)
    lo = np.asarray(a - hi.astype(F32), E4M3)
    return hi, lo


def _quant_w(w):
    """Scale so absmax lands in (112, 224], split hi/lo. Returns
    packed [P, 2, KO, N] planes and inv_scale."""
    m = float(np.abs(w).max())
    s = 2.0 ** math.floor(math.log2(224.0 / m)) if m > 0 else 1.0
    hi, lo = _split_fp8(w.astype(F32) * s)
    K, N = w.shape
    ko = K // P
    pack = np.empty((P, 2, ko, N), E4M3)
    pack[:, 0] = hi.reshape(ko, P, N).transpose(1, 0, 2)
    pack[:, 1] = lo.reshape(ko, P, N).transpose(1, 0, 2)
    return np.ascontiguousarray(pack), 1.0 / s


W1CH = [128] * 4 + [256] * 6 + [512] * 4   # w1 chunk schedule (sum = H)


def _flat_chunks(pack):
    """[P, 2, KO, N] -> [P, 2, KO*N] laid out chunk-major per W1CH so
    each chunk DMA is contiguous on both sides."""
    Pp, two, ko, N = pack.shape
    segs = []
    c0 = 0
    for wc in W1CH:
        segs.append(np.ascontiguousarray(
            pack[:, :, :, c0:c0 + wc]).reshape(Pp, two, ko * wc))
        c0 += wc
    return np.ascontiguousarray(np.concatenate(segs, axis=2))


def _dr_steps(nc, PM, ps, wt, m0, m1, xt_, n0, n1, ko,
              wk0=0, xk0=0, start=True, stop=True, terms=3):
    """3-term fp8 DoubleRow accumulation over `ko` k-tiles (shared path).

    wt: [P, 2, *, M] tile, xt_: [P, 2, *, N] tile, ps: [m1-m0, n1-n0]
    PSUM. Terms (hi,hi), (lo,hi), (hi,lo) share one PSUM scale.
    """
    steps = []
    for k0 in range(0, ko, 2):
        steps.append(((0, k0), (0, k0)))
        steps.append(((1, k0), (0, k0)))
        if terms == 3:
            steps.append(((0, k0), (1, k0)))
    for i, ((wp, wk), (xp, xk)) in enumerate(steps):
        nc.tensor.matmul(ps,
                         wt[:, wp, wk0 + wk:wk0 + wk + 2, m0:m1],
                         xt_[:, xp, xk0 + xk:xk0 + xk + 2, n0:n1],
                         start=(start and i == 0),
                         stop=(stop and i == len(steps) - 1),
                         perf_mode=PM.DoubleRow)


def _dr_plain(nc, PM, ps, wviews, xt_, n0, n1, ko, xk0=0):
    """Accumulate sum of wv.x over plane-selected stationary views.

    wviews: list of [P, 2, M] slicers f(k0) -> stationary AP for k-pair
    k0. xt_: [P, *, N] single-plane moving tile. One PSUM group.
    """
    nst = len(wviews) * (ko // 2)
    i = 0
    for wv in wviews:
        for k0 in range(0, ko, 2):
            nc.tensor.matmul(ps, wv(k0),
                             xt_[:, xk0 + k0:xk0 + k0 + 2, n0:n1],
                             start=(i == 0), stop=(i == nst - 1),
                             perf_mode=PM.DoubleRow)
            i += 1


def _warmup(nc, PM, const, wps, n, width=128):
    """Dummy DR matmuls on a zeroed scratch: keeps the PE continuously busy
    through the p-state ramp while the startup DMAs land."""
    import concourse.mybir as mybir
    f32 = mybir.dt.float32
    fp8 = mybir.dt.float8e4
    # scr is deliberately NOT initialized: whatever SBUF holds is consumed
    # as fp8 into a PSUM tile that is never read, so garbage (even NaN) has
    # no observable effect, and the PE can start ramping immediately
    # instead of waiting ~1.2us on a memset chain.
    scr = const.tile([P, 2, 2, width], fp8, name="warm_scr")
    ps = wps.tile([P, width], f32, tag="warm_ps")
    for i in range(n):
        nc.tensor.matmul(ps, scr[:, :, 0, 0:P], scr[:, :, 1, :],
                         start=(i % 10 == 0),
                         stop=(i % 10 == 9 or i == n - 1),
                         perf_mode=PM.DoubleRow)
    # late write keeps the Tile allocator happy (a tile must have a writer)
    # without making the first matmul wait on a memset chain
    nc.gpsimd.memset(scr, 0.0)


def _build_fused():
    import concourse.mybir as mybir
    import concourse.tile as tile
    from concourse import bacc

    f32 = mybir.dt.float32
    fp8 = mybir.dt.float8e4
    AF = mybir.ActivationFunctionType
    OP = mybir.AluOpType
    PM = mybir.MatmulPerfMode

    nc = bacc.Bacc("TRN2", target_bir_lowering=False, debug=False,
                   num_devices=NCORES)

    xs8a = nc.dram_tensor("xs8a", [P, 2, DKO, XA], fp8, kind="ExternalInput")
    xs8b = nc.dram_tensor("xs8b", [P, 2, DKO, XB], fp8, kind="ExternalInput")
    sw1q = nc.dram_tensor("sw1q", [P, 2, DKO * H], fp8, kind="ExternalInput")
    sb1c = nc.dram_tensor("sb1c", [P, HKO], f32, kind="ExternalInput")
    sw2q = nc.dram_tensor("sw2q", [P, 2, HKO, DIM], fp8, kind="ExternalInput")
    sb2c = nc.dram_tensor("sb2c", [P, DKO], f32, kind="ExternalInput")
    sscal = nc.dram_tensor("sscal", [P, 2], f32, kind="ExternalInput")
    xg8 = nc.dram_tensor("xg8", [P, DKO, CAP], fp8, kind="ExternalInput")
    w1q = nc.dram_tensor("w1q", [P, 2, DKO * H], fp8, kind="ExternalInput")
    b1c = nc.dram_tensor("b1c", [P, HKO], f32, kind="ExternalInput")
    w2q = nc.dram_tensor("w2q", [P, 2, HKO, DIM], fp8, kind="ExternalInput")
    b2c = nc.dram_tensor("b2c", [P, DKO], f32, kind="ExternalInput")
    escal = nc.dram_tensor("escal", [P, 2], f32, kind="ExternalInput")
    sh = nc.dram_tensor("sh", [P, DKO, TS], f32, kind="ExternalOutput")
    eo = nc.dram_tensor("eo", [P, DKO, CAP], f32, kind="ExternalOutput")

    # hm tile -> (chunk index, column offset inside chunk)
    hm2ch = {}
    c0 = 0
    for ci, wc in enumerate(W1CH):
        for hm in range(c0 // P, (c0 + wc) // P):
            hm2ch[hm] = (ci, hm * P - c0)
        c0 += wc

    with tile.TileContext(nc) as tc:
        with (
            tc.tile_pool(name="const", bufs=1) as const,
            tc.tile_pool(name="wpool", bufs=1) as wpool,
            tc.tile_pool(name="gp", bufs=3) as gp,
            tc.tile_pool(name="hp", bufs=1) as hp,
            tc.tile_pool(name="op", bufs=4) as op_,
            tc.tile_pool(name="pA", bufs=3, space="PSUM") as pA,
            tc.tile_pool(name="pB", bufs=4, space="PSUM") as pB,
            tc.tile_pool(name="wps", bufs=1, space="PSUM") as wps,
        ):
            _warmup(nc, PM, const, wps, 140, width=128)
            # ---- startup DMAs on one queue, ordered by first use ----
            # per-chunk w1 tiles: shared w1 now, expert w1 later (in-place)
            wch = []      # (view [P,2,DKO,wc], tile, base, wc)
            base = 0
            for i, wc in enumerate(W1CH):
                t = wpool.tile([P, 2, DKO * wc], fp8, name=f"wch{i}")
                wch.append((t.rearrange("p pl (ko w) -> p pl ko w", ko=DKO),
                            t, base, wc))
                base += DKO * wc
            nc.sync.dma_start(wch[0][1], sw1q[:, :, 0:DKO * W1CH[0]])
            x8a = wpool.tile([P, 2, DKO, XA], fp8)
            nc.sync.dma_start(x8a, xs8a[:, :, :, :])
            x8b = wpool.tile([P, 2, DKO, XB], fp8)
            nc.sync.dma_start(x8b, xs8b[:, :, :, :])
            nc.sync.dma_start(wch[1][1],
                                sw1q[:, :, wch[1][2]:wch[1][2] + DKO * W1CH[1]])
            sb1c_sb = const.tile([P, HKO], f32)
            nc.sync.dma_start(sb1c_sb, sb1c[:, :])
            sscal_sb = const.tile([P, 2], f32)
            nc.sync.dma_start(sscal_sb, sscal[:, :])
            for view, t, b0, wc in wch[2:]:
                nc.sync.dma_start(t, sw1q[:, :, b0:b0 + DKO * wc])
            xg_sb = wpool.tile([P, DKO, CAP], fp8)
            nc.sync.dma_start(xg_sb, xg8[:, :, :])
            b1c_sb = const.tile([P, HKO], f32)
            nc.sync.dma_start(b1c_sb, b1c[:, :])
            escal_sb = const.tile([P, 2], f32)
            nc.sync.dma_start(escal_sb, escal[:, :])
            sb2c_sb = const.tile([P, DKO], f32)
            nc.sync.dma_start(sb2c_sb, sb2c[:, :])
            b2c_sb = const.tile([P, DKO], f32)
            nc.sync.dma_start(b2c_sb, b2c[:, :])
            # sw2 in two 512-col halves: expert w2 reuses them in place
            sw2h = []
            for i in range(2):
                t2 = wpool.tile([P, 2, HKO, 512], fp8, name=f"sw2h{i}")
                nc.sync.dma_start(t2, sw2q[:, :, :, i * 512:(i + 1) * 512])
                sw2h.append(t2)

            # ---- shared expert layer 1 (3-term) ----
            HH = HKO // 2
            h8a = hp.tile([P, 2, HH, TS], fp8, tag="h8a")
            h8b = hp.tile([P, 2, HH, TS], fp8, tag="h8b")
            issued_w1 = set()
            for hm in range(HKO):
                h8t, hr = (h8a, hm) if hm < HH else (h8b, hm - HH)
                ci, off = hm2ch[hm]
                wv = wch[ci][0]
                ps = pA.tile([P, TS], f32, tag="ps1")
                _dr_steps(nc, PM, ps[:, 0:XA], wv, off, off + P,
                          x8a, 0, XA, DKO)
                _dr_steps(nc, PM, ps[:, XA:TS], wv, off, off + P,
                          x8b, 0, XB, DKO)
                g = gp.tile([P, TS], f32, tag="g")
                nc.scalar.activation(g, ps, AF.Gelu,
                                     bias=sb1c_sb[:, hm:hm + 1],
                                     scale=sscal_sb[:, 0:1])
                nc.gpsimd.tensor_copy(h8t[:, 0, hr, :], g)
                nc.vector.scalar_tensor_tensor(
                    h8t[:, 1, hr, :], g, 1.0, h8t[:, 0, hr, :],
                    OP.mult, OP.subtract)
                # chunk fully consumed -> start expert-w1 load into it
                nci, noff = hm2ch.get(hm + 1, (len(W1CH), 0))
                if nci != ci:
                    _, t, b0, wc = wch[ci]
                    nc.sync.dma_start(t, w1q[:, :, b0:b0 + DKO * wc])
                    issued_w1.add(ci)

            # routed h buffers: slice 0 owns a tile; slice 1 reuses h8a
            # (safe: its writes start only after shared L2 consumed h8a)
            h8r0 = hp.tile([P, HKO, RSL[0]], fp8, tag="h8r0")
            h8r1 = h8a[:, :, :, 0:RSL[1]].rearrange(
                "p pl hh n -> p (pl hh) n")

            def r_l1(sl):
                """Routed layer 1 for slice sl; 1-term (+ w1_lo on the
                first HI columns of slice 0). Generator: one hm per step."""
                t0 = sum(RSL[:sl])
                W = RSL[sl]
                h8t = h8r0 if sl == 0 else h8r1
                for hm in range(HKO):
                    ci, off = hm2ch[hm]
                    wv = wch[ci][0]
                    ps = pA.tile([P, TS], f32, tag="ps1")
                    if sl == 0 and HI > 0:
                        _dr_plain(nc, PM, ps[:, 0:HI],
                                  [lambda k0, wv=wv, off=off:
                                   wv[:, 0, k0:k0 + 2, off:off + P],
                                   lambda k0, wv=wv, off=off:
                                   wv[:, 1, k0:k0 + 2, off:off + P]],
                                  xg_sb, t0, t0 + HI, DKO)
                        _dr_plain(nc, PM, ps[:, HI:W],
                                  [lambda k0, wv=wv, off=off:
                                   wv[:, 0, k0:k0 + 2, off:off + P]],
                                  xg_sb, t0 + HI, t0 + W, DKO)
                    else:
                        _dr_plain(nc, PM, ps[:, 0:W],
                                  [lambda k0, wv=wv, off=off:
                                   wv[:, 0, k0:k0 + 2, off:off + P]],
                                  xg_sb, t0, t0 + W, DKO)
                    g = gp.tile([P, TS], f32, tag="g")
                    nc.scalar.activation(g[:, 0:W], ps[:, 0:W], AF.Gelu,
                                         bias=b1c_sb[:, hm:hm + 1],
                                         scale=escal_sb[:, 0:1])
                    nc.gpsimd.tensor_copy(h8t[:, hm, 0:W], g[:, 0:W])
                    yield

            def s_l2():
                """Shared layer 2 (3-term). Generator: one dm per step."""
                for dm in range(DKO):
                    w2t = sw2h[dm // 4]
                    m0 = (dm % 4) * P
                    ps2 = pB.tile([P, TS], f32, tag="ps2")
                    _dr_steps(nc, PM, ps2, w2t, m0, m0 + P,
                              h8a, 0, TS, HH, start=True, stop=False)
                    _dr_steps(nc, PM, ps2, w2t, m0, m0 + P,
                              h8b, 0, TS, HH, wk0=HH, start=False, stop=True)
                    o_sb = op_.tile([P, TS], f32, tag="o_sb")
                    nc.scalar.activation(o_sb, ps2, AF.Identity,
                                         bias=sb2c_sb[:, dm:dm + 1],
                                         scale=sscal_sb[:, 1:2])
                    nc.sync.dma_start(sh[:, dm, :], o_sb)
                    if dm == 3 or dm == DKO - 1:
                        # half fully consumed -> expert-w2 load into it
                        i = dm // 4
                        nc.sync.dma_start(
                            sw2h[i], w2q[:, :, :, i * 512:(i + 1) * 512])
                    yield

            def r_l2(sl, tail_split=False):
                """Routed layer 2 for slice sl; 1-term (+ w2_lo on the
                first HI columns of slice 0). Generator: one dm per step.
                With tail_split, the final dm runs as 128-col PSUM groups
                with pipelined act+DMA so the end-of-kernel drain is one
                small chunk instead of a full 512-col store."""
                t0 = sum(RSL[:sl])
                W = RSL[sl]
                h8t = h8r0 if sl == 0 else h8r1
                for dm in range(DKO):
                    w2t = sw2h[dm // 4]
                    m0 = (dm % 4) * P
                    if tail_split and dm == DKO - 1:
                        # last dm as two pieces so the end-of-kernel drain
                        # is a 128-col store instead of a 512-col one
                        for c0, cw in ((0, W - 64), (W - 64, 64)):
                            psg = pB.tile([P, TS], f32, tag="ps2")
                            _dr_plain(nc, PM, psg[:, 0:cw],
                                      [lambda k0, w2t=w2t, m0=m0:
                                       w2t[:, 0, k0:k0 + 2, m0:m0 + P]],
                                      h8t, c0, c0 + cw, HKO)
                            od = op_.tile([P, TS], f32, tag="o_sb")
                            nc.scalar.activation(
                                od[:, 0:cw], psg[:, 0:cw], AF.Identity,
                                bias=b2c_sb[:, dm:dm + 1],
                                scale=escal_sb[:, 1:2])
                            # the two pieces go out on different queues so
                            # their DGE configs and transfers overlap
                            dmae = nc.scalar if c0 == 0 else nc.sync
                            dmae.dma_start(
                                eo[:, dm, t0 + c0:t0 + c0 + cw], od[:, 0:cw])
                        yield
                        continue
                    ps2 = pB.tile([P, TS], f32, tag="ps2")
                    if sl == 0 and HI > 0:
                        _dr_plain(nc, PM, ps2[:, 0:HI],
                                  [lambda k0, w2t=w2t, m0=m0:
                                   w2t[:, 0, k0:k0 + 2, m0:m0 + P],
                                   lambda k0, w2t=w2t, m0=m0:
                                   w2t[:, 1, k0:k0 + 2, m0:m0 + P]],
                                  h8t, 0, HI, HKO)
                        _dr_plain(nc, PM, ps2[:, HI:W],
                                  [lambda k0, w2t=w2t, m0=m0:
                                   w2t[:, 0, k0:k0 + 2, m0:m0 + P]],
                                  h8t, HI, W, HKO)
                    else:
                        _dr_plain(nc, PM, ps2[:, 0:W],
                                  [lambda k0, w2t=w2t, m0=m0:
                                   w2t[:, 0, k0:k0 + 2, m0:m0 + P]],
                                  h8t, 0, W, HKO)
                    od = op_.tile([P, TS], f32, tag="o_sb")
                    nc.scalar.activation(od[:, 0:W], ps2[:, 0:W], AF.Identity,
                                         bias=b2c_sb[:, dm:dm + 1],
                                         scale=escal_sb[:, 1:2])
                    nc.sync.dma_start(eo[:, dm, t0:t0 + W], od[:, 0:W])
                    yield

            # ---- emission schedule ----
            g1 = r_l1(0)
            for _ in range(4):        # fill the shared L1->L2 gelu gap
                next(g1, None)
            for _ in s_l2():
                for _ in range(4):
                    next(g1, None)
            for _ in g1:
                pass
            g2 = r_l1(1)
            for _ in r_l2(0):
                for _ in range(8):     # front-loaded so h8r1 is ready early
                    next(g2, None)
            for _ in g2:
                pass
            for _ in r_l2(1, tail_split=True):
                pass

    nc.finalize()
    return nc


def _get(name, builder):
    if name not in _nc_cache:
        _nc_cache[name] = builder()
    return _nc_cache[name]


def _run_spmd(nc, in_maps):
    from concourse.bass_utils import run_bass_kernel_spmd
    return run_bass_kernel_spmd(nc, in_maps, core_ids=list(range(NCORES)))


def _gelu_np(v):
    from scipy.special import erf
    return 0.5 * v * (1.0 + erf(v / np.sqrt(2.0)))


def kernel(x, router_w, router_b, w1, b1, w2, b2, sw1, sb1, sw2, sb2):
    x = np.asarray(x, F32)
    x2 = x.reshape(T, DIM)
    xt = np.ascontiguousarray(x2.T)                          # [DIM, T]
    xhi, xlo = _split_fp8(xt)
    x8p = np.empty((P, 2, DKO, T), E4M3)
    x8p[:, 0] = xhi.reshape(DKO, P, T).transpose(1, 0, 2)
    x8p[:, 1] = xlo.reshape(DKO, P, T).transpose(1, 0, 2)

    # ---- host router: exact fp32, matching the reference math ----
    logits = x2 @ np.asarray(router_w, F32) + np.asarray(router_b, F32)
    ex = np.exp(logits - logits.max(axis=-1, keepdims=True))
    probs = ex / ex.sum(axis=-1, keepdims=True)
    order = np.argsort(-probs, axis=-1, kind="stable")
    ar = np.arange(T)
    cw = np.zeros((T, E), F32)
    cw[ar, order[:, 0]] = probs[ar, order[:, 0]]
    cw[ar, order[:, 1]] = probs[ar, order[:, 1]]

    sw1q, s1 = _quant_w(np.asarray(sw1, F32))
    sw1q = _flat_chunks(sw1q)
    sw2q, s2 = _quant_w(np.asarray(sw2, F32))
    sb1cp = np.ascontiguousarray(np.asarray(sb1, F32).reshape(HKO, P).T)
    sb2cp = np.ascontiguousarray(np.asarray(sb2, F32).reshape(DKO, P).T)
    sscal = np.ascontiguousarray(np.tile(np.array([[s1, s2]], F32), (P, 1)))

    maps = []
    sels = []
    csorted = []
    overflow = []
    for e in range(E):
        sel = np.nonzero(cw[:, e])[0]
        c = cw[sel, e]
        o = np.argsort(-c, kind="stable")
        sel, c = sel[o], c[o]
        if len(sel) > CAP:
            overflow.append((e, sel[CAP:]))
            sel, c = sel[:CAP], c[:CAP]
        sels.append(sel)
        csorted.append(c)
        npad = CAP - len(sel)
        selp = np.concatenate([sel, np.zeros(npad, np.int64)])
        w1qe, e1 = _quant_w(np.asarray(w1[e], F32))
        w1qe = _flat_chunks(w1qe)
        w2qe, e2 = _quant_w(np.asarray(w2[e], F32))
        t0 = e * TS
        maps.append(dict(
            xs8a=np.ascontiguousarray(x8p[:, :, :, t0:t0 + XA]),
            xs8b=np.ascontiguousarray(x8p[:, :, :, t0 + XA:t0 + TS]),
            sw1q=sw1q, sb1c=sb1cp, sw2q=sw2q, sb2c=sb2cp, sscal=sscal,
            xg8=np.ascontiguousarray(x8p[:, 0][:, :, selp]),
            w1q=w1qe,
            b1c=np.ascontiguousarray(
                np.asarray(b1[e], F32).reshape(HKO, P).T),
            w2q=w2qe,
            b2c=np.ascontiguousarray(
                np.asarray(b2[e], F32).reshape(DKO, P).T),
            escal=np.ascontiguousarray(
                np.tile(np.array([[e1, e2]], F32), (P, 1)))))
    res = _run_spmd(_get("fused", _build_fused), maps)

    out = np.empty((T, DIM), F32)
    for j, r in enumerate(res.results):
        # sh [P, DKO, TS]: value (ki, dm, t) = shared[dim dm*128+ki, tok]
        out[j * TS:(j + 1) * TS] = \
            r["sh"].transpose(2, 1, 0).reshape(TS, DIM)
    for e, r in enumerate(res.results):
        n = len(sels[e])
        ye = r["eo"].transpose(2, 1, 0).reshape(CAP, DIM)
        out[sels[e]] += ye[:n] * csorted[e][:, None]
    for e, toks in overflow:
        xe = x2[toks]
        he = _gelu_np(xe @ np.asarray(w1[e], F32) + np.asarray(b1[e], F32))
        ye = he @ np.asarray(w2[e], F32) + np.asarray(b2[e], F32)
        out[toks] += ye * cw[toks, e:e + 1]

    return out.reshape(2, 2048, DIM)


# revision 48
# speedup vs baseline: 1.0322x; 1.0322x over previous
"""Trainium2 Bass kernel for an 8-expert top-2 MoE layer (+ shared expert).

Single-NEFF fused design over 8 NeuronCores. The router (0.03% of FLOPs)
and the top-2 dispatch/combine run on the host in exact fp32; each core
runs ONE program computing

  - the shared-expert FFN for its 512-token slice (fp8 e4m3 residual
    matmuls, 3 terms/layer: w_hi.x_hi + w_lo.x_hi + w_hi.x_lo), and
  - expert j's FFN over C=1024 routed tokens, gathered by the host and
    sorted by combine weight descending. Slots use a precision ladder:
    the 128 highest-weight slots get a 2-term scheme per layer (both
    weight planes vs activation-hi), the rest run 1-term (hi-only).
    Routed activations never need a residual split on device.

All matmuls are fp8(e4m3) DoubleRow (two 128-deep k-tiles per
instruction, fp32 PSUM). Per-expert token lists beyond the capacity C
fall back to an exact host fp32 path (the dropped tokens are the
lowest-combine-weight ones, so this also improves accuracy).

Perf structure: one NEFF = one p-state ramp + one startup/drain. Weight
buffers are reused in place: the shared-expert w1 chunk tiles are
overwritten by expert w1 chunks as soon as shared layer 1 has consumed
them (fine-grained per-chunk WAR), likewise sw2 -> w2 halves behind
shared layer 2. Routed layer-1 tiles are interleaved into the shared
layer-2 (and routed layer-2) instruction stream so the PE never waits on
a gelu chain; dummy warm-up matmuls carry the PE through its p-state
ramp during the startup DMAs. The combine-weight multiply and the
scatter-add happen on the host.
"""

import sys

if "/opt/trn_rl_repo" not in sys.path:
    sys.path.insert(0, "/opt/trn_rl_repo")

import math

import numpy as np
import ml_dtypes

DIM = 1024
E = 8
H = 4096
T = 4096  # B*S = 2*2048 tokens
NCORES = 8
P = 128
DKO = DIM // P     # 8 k-subtiles over dim
HKO = H // P       # 32 k-subtiles over hidden
TS = T // NCORES   # 512 shared tokens per core
CAP = 1024         # routed capacity per expert (overflow -> exact host path)
HI = 96           # slots with the 2-term scheme (highest combine weight)
RSL = [512, 512]   # routed token slices
XA = 448           # shared-x split: big piece lands first, its L1 compute
XB = TS - XA       # covers the small piece's arrival
WARMN = 105        # warm-up matmul count (fills the startup DMA latency)
G2IL = 6           # r_l1(1) tiles interleaved per r_l2(0) dm step
TAILW = 160         # width of the final store piece

E4M3 = ml_dtypes.float8_e4m3
F32 = np.float32

_nc_cache = {}


def _split_fp8(a):
    """e4m3 hi/lo residual split (same scale for both planes)."""
    hi = np.asarray(a, E4M3)
    lo = np.asarray(a - hi.astype(F32), E4M3)
    return hi, lo


def _quant_w(w):
    """Scale so absmax lands in (112, 224], split hi/lo. Returns
    packed [P, 2, KO, N] planes and inv_scale."""
    m = float(np.abs(w).max())
    s = 2.0 ** math.floor(math.log2(224.0 / m)) if m > 0 else 1.0
    hi, lo = _split_fp8(w.astype(F32) * s)
    K, N = w.shape
    ko = K // P
    pack = np.empty((P, 2, ko, N), E4M3)
    pack[:, 0] = hi.reshape(ko, P, N).transpose(1, 0, 2)
    pack[:, 1] = lo.reshape(ko, P, N).transpose(1, 0, 2)
    return np.ascontiguousarray(pack), 1.0 / s


W1CH = [128] * 4 + [256] * 6 + [512] * 4   # w1 chunk schedule (sum = H)


def _flat_chunks(pack):
    """[P, 2, KO, N] -> [P, 2, KO*N] laid out chunk-major per W1CH so
    each chunk DMA is contiguous on both sides."""
    Pp, two, ko, N = pack.shape
    segs = []
    c0 = 0
    for wc in W1CH:
        segs.append(np.ascontiguousarray(
            pack[:, :, :, c0:c0 + wc]).reshape(Pp, two, ko * wc))
        c0 += wc
    return np.ascontiguousarray(np.concatenate(segs, axis=2))


def _dr_steps(nc, PM, ps, wt, m0, m1, xt_, n0, n1, ko,
              wk0=0, xk0=0, start=True, stop=True, terms=3):
    """3-term fp8 DoubleRow accumulation over `ko` k-tiles (shared path).

    wt: [P, 2, *, M] tile, xt_: [P, 2, *, N] tile, ps: [m1-m0, n1-n0]
    PSUM. Terms (hi,hi), (lo,hi), (hi,lo) share one PSUM scale.
    """
    steps = []
    for k0 in range(0, ko, 2):
        steps.append(((0, k0), (0, k0)))
        steps.append(((1, k0), (0, k0)))
        if terms == 3:
            steps.append(((0, k0), (1, k0)))
    for i, ((wp, wk), (xp, xk)) in enumerate(steps):
        nc.tensor.matmul(ps,
                         wt[:, wp, wk0 + wk:wk0 + wk + 2, m0:m1],
                         xt_[:, xp, xk0 + xk:xk0 + xk + 2, n0:n1],
                         start=(start and i == 0),
                         stop=(stop and i == len(steps) - 1),
                         perf_mode=PM.DoubleRow)


def _dr_plain(nc, PM, ps, wviews, xt_, n0, n1, ko, xk0=0):
    """Accumulate sum of wv.x over plane-selected stationary views.

    wviews: list of [P, 2, M] slicers f(k0) -> stationary AP for k-pair
    k0. xt_: [P, *, N] single-plane moving tile. One PSUM group.
    """
    nst = len(wviews) * (ko // 2)
    i = 0
    for wv in wviews:
        for k0 in range(0, ko, 2):
            nc.tensor.matmul(ps, wv(k0),
                             xt_[:, xk0 + k0:xk0 + k0 + 2, n0:n1],
                             start=(i == 0), stop=(i == nst - 1),
                             perf_mode=PM.DoubleRow)
            i += 1


def _warmup(nc, PM, const, wps, n, width=128):
    """Dummy DR matmuls on a zeroed scratch: keeps the PE continuously busy
    through the p-state ramp while the startup DMAs land."""
    import concourse.mybir as mybir
    f32 = mybir.dt.float32
    fp8 = mybir.dt.float8e4
    # scr is deliberately NOT initialized: whatever SBUF holds is consumed
    # as fp8 into a PSUM tile that is never read, so garbage (even NaN) has
    # no observable effect, and the PE can start ramping immediately
    # instead of waiting ~1.2us on a memset chain.
    scr = const.tile([P, 2, 2, width], fp8, name="warm_scr")
    ps = wps.tile([P, width], f32, tag="warm_ps")
    for i in range(n):
        nc.tensor.matmul(ps, scr[:, :, 0, 0:P], scr[:, :, 1, :],
                         start=(i % 10 == 0),
                         stop=(i % 10 == 9 or i == n - 1),
                         perf_mode=PM.DoubleRow)
    # late write keeps the Tile allocator happy (a tile must have a writer)
    # without making the first matmul wait on a memset chain
    nc.gpsimd.memset(scr, 0.0)


def _build_fused():
    import concourse.mybir as mybir
    import concourse.tile as tile
    from concourse import bacc

    f32 = mybir.dt.float32
    fp8 = mybir.dt.float8e4
    AF = mybir.ActivationFunctionType
    OP = mybir.AluOpType
    PM = mybir.MatmulPerfMode

    nc = bacc.Bacc("TRN2", target_bir_lowering=False, debug=False,
                   num_devices=NCORES)

    xs8a = nc.dram_tensor("xs8a", [P, 2, DKO, XA], fp8, kind="ExternalInput")
    xs8b = nc.dram_tensor("xs8b", [P, 2, DKO, XB], fp8, kind="ExternalInput")
    sw1q = nc.dram_tensor("sw1q", [P, 2, DKO * H], fp8, kind="ExternalInput")
    sb1c = nc.dram_tensor("sb1c", [P, HKO], f32, kind="ExternalInput")
    sw2q = nc.dram_tensor("sw2q", [P, 2, HKO, DIM], fp8, kind="ExternalInput")
    sb2c = nc.dram_tensor("sb2c", [P, DKO], f32, kind="ExternalInput")
    sscal = nc.dram_tensor("sscal", [P, 2], f32, kind="ExternalInput")
    xg8 = nc.dram_tensor("xg8", [P, DKO, CAP], fp8, kind="ExternalInput")
    w1q = nc.dram_tensor("w1q", [P, 2, DKO * H], fp8, kind="ExternalInput")
    b1c = nc.dram_tensor("b1c", [P, HKO], f32, kind="ExternalInput")
    w2q = nc.dram_tensor("w2q", [P, 2, HKO, DIM], fp8, kind="ExternalInput")
    b2c = nc.dram_tensor("b2c", [P, DKO], f32, kind="ExternalInput")
    escal = nc.dram_tensor("escal", [P, 2], f32, kind="ExternalInput")
    sh = nc.dram_tensor("sh", [P, DKO, TS], f32, kind="ExternalOutput")
    eo = nc.dram_tensor("eo", [P, DKO, CAP], f32, kind="ExternalOutput")

    # hm tile -> (chunk index, column offset inside chunk)
    hm2ch = {}
    c0 = 0
    for ci, wc in enumerate(W1CH):
        for hm in range(c0 // P, (c0 + wc) // P):
            hm2ch[hm] = (ci, hm * P - c0)
        c0 += wc

    with tile.TileContext(nc) as tc:
        with (
            tc.tile_pool(name="const", bufs=1) as const,
            tc.tile_pool(name="wpool", bufs=1) as wpool,
            tc.tile_pool(name="gp", bufs=3) as gp,
            tc.tile_pool(name="hp", bufs=1) as hp,
            tc.tile_pool(name="op", bufs=4) as op_,
            tc.tile_pool(name="pA", bufs=3, space="PSUM") as pA,
            tc.tile_pool(name="pB", bufs=4, space="PSUM") as pB,
            tc.tile_pool(name="wps", bufs=1, space="PSUM") as wps,
        ):
            _warmup(nc, PM, const, wps, WARMN, width=128)
            # ---- startup DMAs on one queue, ordered by first use ----
            # per-chunk w1 tiles: shared w1 now, expert w1 later (in-place)
            wch = []      # (view [P,2,DKO,wc], tile, base, wc)
            base = 0
            for i, wc in enumerate(W1CH):
                t = wpool.tile([P, 2, DKO * wc], fp8, name=f"wch{i}")
                wch.append((t.rearrange("p pl (ko w) -> p pl ko w", ko=DKO),
                            t, base, wc))
                base += DKO * wc
            nc.sync.dma_start(wch[0][1], sw1q[:, :, 0:DKO * W1CH[0]])
            x8a = wpool.tile([P, 2, DKO, XA], fp8)
            nc.sync.dma_start(x8a, xs8a[:, :, :, :])
            x8b = wpool.tile([P, 2, DKO, XB], fp8)
            nc.sync.dma_start(x8b, xs8b[:, :, :, :])
            nc.sync.dma_start(wch[1][1],
                                sw1q[:, :, wch[1][2]:wch[1][2] + DKO * W1CH[1]])
            sb1c_sb = const.tile([P, HKO], f32)
            nc.sync.dma_start(sb1c_sb, sb1c[:, :])
            sscal_sb = const.tile([P, 2], f32)
            nc.sync.dma_start(sscal_sb, sscal[:, :])
            for view, t, b0, wc in wch[2:]:
                nc.sync.dma_start(t, sw1q[:, :, b0:b0 + DKO * wc])
            xg_sb = wpool.tile([P, DKO, CAP], fp8)
            nc.sync.dma_start(xg_sb, xg8[:, :, :])
            b1c_sb = const.tile([P, HKO], f32)
            nc.sync.dma_start(b1c_sb, b1c[:, :])
            escal_sb = const.tile([P, 2], f32)
            nc.sync.dma_start(escal_sb, escal[:, :])
            sb2c_sb = const.tile([P, DKO], f32)
            nc.sync.dma_start(sb2c_sb, sb2c[:, :])
            b2c_sb = const.tile([P, DKO], f32)
            nc.sync.dma_start(b2c_sb, b2c[:, :])
            # sw2 in two 512-col halves: expert w2 reuses them in place
            sw2h = []
            for i in range(2):
                t2 = wpool.tile([P, 2, HKO, 512], fp8, name=f"sw2h{i}")
                nc.sync.dma_start(t2, sw2q[:, :, :, i * 512:(i + 1) * 512])
                sw2h.append(t2)

            # ---- shared expert layer 1 (3-term) ----
            HH = HKO // 2
            h8a = hp.tile([P, 2, HH, TS], fp8, tag="h8a")
            h8b = hp.tile([P, 2, HH, TS], fp8, tag="h8b")
            issued_w1 = set()
            for hm in range(HKO):
                h8t, hr = (h8a, hm) if hm < HH else (h8b, hm - HH)
                ci, off = hm2ch[hm]
                wv = wch[ci][0]
                ps = pA.tile([P, TS], f32, tag="ps1")
                _dr_steps(nc, PM, ps[:, 0:XA], wv, off, off + P,
                          x8a, 0, XA, DKO)
                _dr_steps(nc, PM, ps[:, XA:TS], wv, off, off + P,
                          x8b, 0, XB, DKO)
                g = gp.tile([P, TS], f32, tag="g")
                nc.scalar.activation(g, ps, AF.Gelu,
                                     bias=sb1c_sb[:, hm:hm + 1],
                                     scale=sscal_sb[:, 0:1])
                nc.gpsimd.tensor_copy(h8t[:, 0, hr, :], g)
                nc.vector.scalar_tensor_tensor(
                    h8t[:, 1, hr, :], g, 1.0, h8t[:, 0, hr, :],
                    OP.mult, OP.subtract)
                # chunk fully consumed -> start expert-w1 load into it
                nci, noff = hm2ch.get(hm + 1, (len(W1CH), 0))
                if nci != ci:
                    _, t, b0, wc = wch[ci]
                    nc.sync.dma_start(t, w1q[:, :, b0:b0 + DKO * wc])
                    issued_w1.add(ci)

            # routed h buffers: slice 0 owns a tile; slice 1 reuses h8a
            # (safe: its writes start only after shared L2 consumed h8a)
            h8r0 = hp.tile([P, HKO, RSL[0]], fp8, tag="h8r0")
            h8r1 = h8a[:, :, :, 0:RSL[1]].rearrange(
                "p pl hh n -> p (pl hh) n")

            def r_l1(sl):
                """Routed layer 1 for slice sl; 1-term (+ w1_lo on the
                first HI columns of slice 0). Generator: one hm per step."""
                t0 = sum(RSL[:sl])
                W = RSL[sl]
                h8t = h8r0 if sl == 0 else h8r1
                for hm in range(HKO):
                    ci, off = hm2ch[hm]
                    wv = wch[ci][0]
                    ps = pA.tile([P, TS], f32, tag="ps1")
                    if sl == 0 and HI > 0:
                        _dr_plain(nc, PM, ps[:, 0:HI],
                                  [lambda k0, wv=wv, off=off:
                                   wv[:, 0, k0:k0 + 2, off:off + P],
                                   lambda k0, wv=wv, off=off:
                                   wv[:, 1, k0:k0 + 2, off:off + P]],
                                  xg_sb, t0, t0 + HI, DKO)
                        _dr_plain(nc, PM, ps[:, HI:W],
                                  [lambda k0, wv=wv, off=off:
                                   wv[:, 0, k0:k0 + 2, off:off + P]],
                                  xg_sb, t0 + HI, t0 + W, DKO)
                    else:
                        _dr_plain(nc, PM, ps[:, 0:W],
                                  [lambda k0, wv=wv, off=off:
                                   wv[:, 0, k0:k0 + 2, off:off + P]],
                                  xg_sb, t0, t0 + W, DKO)
                    g = gp.tile([P, TS], f32, tag="g")
                    nc.scalar.activation(g[:, 0:W], ps[:, 0:W], AF.Gelu,
                                         bias=b1c_sb[:, hm:hm + 1],
                                         scale=escal_sb[:, 0:1])
                    nc.gpsimd.tensor_copy(h8t[:, hm, 0:W], g[:, 0:W])
                    yield

            def s_l2():
                """Shared layer 2 (3-term). Generator: one dm per step."""
                for dm in range(DKO):
                    w2t = sw2h[dm // 4]
                    m0 = (dm % 4) * P
                    ps2 = pB.tile([P, TS], f32, tag="ps2")
                    _dr_steps(nc, PM, ps2, w2t, m0, m0 + P,
                              h8a, 0, TS, HH, start=True, stop=False)
                    _dr_steps(nc, PM, ps2, w2t, m0, m0 + P,
                              h8b, 0, TS, HH, wk0=HH, start=False, stop=True)
                    o_sb = op_.tile([P, TS], f32, tag="o_sb")
                    nc.scalar.activation(o_sb, ps2, AF.Identity,
                                         bias=sb2c_sb[:, dm:dm + 1],
                                         scale=sscal_sb[:, 1:2])
                    nc.sync.dma_start(sh[:, dm, :], o_sb)
                    if dm == 3 or dm == DKO - 1:
                        # half fully consumed -> expert-w2 load into it
                        i = dm // 4
                        nc.sync.dma_start(
                            sw2h[i], w2q[:, :, :, i * 512:(i + 1) * 512])
                    yield

            def r_l2(sl, tail_split=False):
                """Routed layer 2 for slice sl; 1-term (+ w2_lo on the
                first HI columns of slice 0). Generator: one dm per step.
                With tail_split, the final dm runs as 128-col PSUM groups
                with pipelined act+DMA so the end-of-kernel drain is one
                small chunk instead of a full 512-col store."""
                t0 = sum(RSL[:sl])
                W = RSL[sl]
                h8t = h8r0 if sl == 0 else h8r1
                for dm in range(DKO):
                    w2t = sw2h[dm // 4]
                    m0 = (dm % 4) * P
                    if tail_split and dm == DKO - 1:
                        # last dm as two pieces so the end-of-kernel drain
                        # is a 128-col store instead of a 512-col one
                        for c0, cw in ((0, W - TAILW), (W - TAILW, TAILW)):
                            psg = pB.tile([P, TS], f32, tag="ps2")
                            _dr_plain(nc, PM, psg[:, 0:cw],
                                      [lambda k0, w2t=w2t, m0=m0:
                                       w2t[:, 0, k0:k0 + 2, m0:m0 + P]],
                                      h8t, c0, c0 + cw, HKO)
                            od = op_.tile([P, TS], f32, tag="o_sb")
                            nc.scalar.activation(
                                od[:, 0:cw], psg[:, 0:cw], AF.Identity,
                                bias=b2c_sb[:, dm:dm + 1],
                                scale=escal_sb[:, 1:2])
                            # the two pieces go out on different queues so
                            # their DGE configs and transfers overlap
                            dmae = nc.scalar if c0 == 0 else nc.sync
                            dmae.dma_start(
                                eo[:, dm, t0 + c0:t0 + c0 + cw], od[:, 0:cw])
                        yield
                        continue
                    ps2 = pB.tile([P, TS], f32, tag="ps2")
                    if sl == 0 and HI > 0:
                        _dr_plain(nc, PM, ps2[:, 0:HI],
                                  [lambda k0, w2t=w2t, m0=m0:
                                   w2t[:, 0, k0:k0 + 2, m0:m0 + P],
                                   lambda k0, w2t=w2t, m0=m0:
                                   w2t[:, 1, k0:k0 + 2, m0:m0 + P]],
                                  h8t, 0, HI, HKO)
                        _dr_plain(nc, PM, ps2[:, HI:W],
                                  [lambda k0, w2t=w2t, m0=m0:
                                   w2t[:, 0, k0:k0 + 2, m0:m0 + P]],
                                  h8t, HI, W, HKO)
                    else:
                        _dr_plain(nc, PM, ps2[:, 0:W],
                                  [lambda k0, w2t=w2t, m0=m0:
                                   w2t[:, 0, k0:k0 + 2, m0:m0 + P]],
                                  h8t, 0, W, HKO)
                    od = op_.tile([P, TS], f32, tag="o_sb")
                    nc.scalar.activation(od[:, 0:W], ps2[:, 0:W], AF.Identity,
                                         bias=b2c_sb[:, dm:dm + 1],
                                         scale=escal_sb[:, 1:2])
                    nc.scalar.dma_start(eo[:, dm, t0:t0 + W], od[:, 0:W])
                    yield

            # ---- emission schedule ----
            g1 = r_l1(0)
            for _ in range(4):        # fill the shared L1->L2 gelu gap
                next(g1, None)
            for _ in s_l2():
                for _ in range(4):
                    next(g1, None)
            for _ in g1:
                pass
            g2 = r_l1(1)
            for _ in r_l2(0):
                for _ in range(G2IL):  # front-loaded so h8r1 is ready early
                    next(g2, None)
            for _ in g2:
                pass
            for _ in r_l2(1, tail_split=True):
                pass

    nc.finalize()
    return nc


def _get(name, builder):
    if name not in _nc_cache:
        _nc_cache[name] = builder()
    return _nc_cache[name]


def _run_spmd(nc, in_maps):
    from concourse.bass_utils import run_bass_kernel_spmd
    return run_bass_kernel_spmd(nc, in_maps, core_ids=list(range(NCORES)))


def _gelu_np(v):
    from scipy.special import erf
    return 0.5 * v * (1.0 + erf(v / np.sqrt(2.0)))


def kernel(x, router_w, router_b, w1, b1, w2, b2, sw1, sb1, sw2, sb2):
    x = np.asarray(x, F32)
    x2 = x.reshape(T, DIM)
    xt = np.ascontiguousarray(x2.T)                          # [DIM, T]
    xhi, xlo = _split_fp8(xt)
    x8p = np.empty((P, 2, DKO, T), E4M3)
    x8p[:, 0] = xhi.reshape(DKO, P, T).transpose(1, 0, 2)
    x8p[:, 1] = xlo.reshape(DKO, P, T).transpose(1, 0, 2)

    # ---- host router: exact fp32, matching the reference math ----
    logits = x2 @ np.asarray(router_w, F32) + np.asarray(router_b, F32)
    ex = np.exp(logits - logits.max(axis=-1, keepdims=True))
    probs = ex / ex.sum(axis=-1, keepdims=True)
    order = np.argsort(-probs, axis=-1, kind="stable")
    ar = np.arange(T)
    cw = np.zeros((T, E), F32)
    cw[ar, order[:, 0]] = probs[ar, order[:, 0]]
    cw[ar, order[:, 1]] = probs[ar, order[:, 1]]

    sw1q, s1 = _quant_w(np.asarray(sw1, F32))
    sw1q = _flat_chunks(sw1q)
    sw2q, s2 = _quant_w(np.asarray(sw2, F32))
    sb1cp = np.ascontiguousarray(np.asarray(sb1, F32).reshape(HKO, P).T)
    sb2cp = np.ascontiguousarray(np.asarray(sb2, F32).reshape(DKO, P).T)
    sscal = np.ascontiguousarray(np.tile(np.array([[s1, s2]], F32), (P, 1)))

    maps = []
    sels = []
    csorted = []
    overflow = []
    for e in range(E):
        sel = np.nonzero(cw[:, e])[0]
        c = cw[sel, e]
        o = np.argsort(-c, kind="stable")
        sel, c = sel[o], c[o]
        if len(sel) > CAP:
            overflow.append((e, sel[CAP:]))
            sel, c = sel[:CAP], c[:CAP]
        sels.append(sel)
        csorted.append(c)
        npad = CAP - len(sel)
        selp = np.concatenate([sel, np.zeros(npad, np.int64)])
        w1qe, e1 = _quant_w(np.asarray(w1[e], F32))
        w1qe = _flat_chunks(w1qe)
        w2qe, e2 = _quant_w(np.asarray(w2[e], F32))
        t0 = e * TS
        maps.append(dict(
            xs8a=np.ascontiguousarray(x8p[:, :, :, t0:t0 + XA]),
            xs8b=np.ascontiguousarray(x8p[:, :, :, t0 + XA:t0 + TS]),
            sw1q=sw1q, sb1c=sb1cp, sw2q=sw2q, sb2c=sb2cp, sscal=sscal,
            xg8=np.ascontiguousarray(x8p[:, 0][:, :, selp]),
            w1q=w1qe,
            b1c=np.ascontiguousarray(
                np.asarray(b1[e], F32).reshape(HKO, P).T),
            w2q=w2qe,
            b2c=np.ascontiguousarray(
                np.asarray(b2[e], F32).reshape(DKO, P).T),
            escal=np.ascontiguousarray(
                np.tile(np.array([[e1, e2]], F32), (P, 1)))))
    res = _run_spmd(_get("fused", _build_fused), maps)

    out = np.empty((T, DIM), F32)
    for j, r in enumerate(res.results):
        # sh [P, DKO, TS]: value (ki, dm, t) = shared[dim dm*128+ki, tok]
        out[j * TS:(j + 1) * TS] = \
            r["sh"].transpose(2, 1, 0).reshape(TS, DIM)
    for e, r in enumerate(res.results):
        n = len(sels[e])
        ye = r["eo"].transpose(2, 1, 0).reshape(CAP, DIM)
        out[sels[e]] += ye[:n] * csorted[e][:, None]
    for e, toks in overflow:
        xe = x2[toks]
        he = _gelu_np(xe @ np.asarray(w1[e], F32) + np.asarray(b1[e], F32))
        ye = he @ np.asarray(w2[e], F32) + np.asarray(b2[e], F32)
        out[toks] += ye * cw[toks, e:e + 1]

    return out.reshape(2, 2048, DIM)
